# revision 8
# baseline (speedup 1.0000x reference)
"""Trainium2 Bass kernel for equivariant multihead attention.

Math (per batch b, query point i, coset s1, channel c):
    logit[j,s2] = sum_g pairwise_g[b,i,j,s1,s2,g]*w_g[c,g]
                  + w_y[c,0]*y[b,j,s2,c] + w_y[c,1]*y[b,i,s1,c] + b_g[c] + b_y[c]
    att = exp(logit)*mask[b,j,s2];  att /= sum_{j,s2} att
    out = (y[b,i,s1,c] + sum_{j,s2} att*y[b,j,s2,c]) * mask[b,i,s1]  @ w_lin.T

The query-side term and the biases are constant over the key dims (j,s2) and
cancel in the normalization, so they are dropped.  The key-side factor
exp(w_y[c,0]*y)*mask is folded INTO the exponent: with
    L[(s2,c), j] = w_y0[c]*y[b,j,s2,c] + log(mask[b,j,s2])   (pad -> -240)
the unnormalized attention is E' = exp(sum_g pg*w_g + L) directly.

TWO query points (a "pair") are packed per output tile: partition is
(i2, s1, c) = 2*8*8 = 128; the free dim is the mask-compacted key list
(s2, j), padded to padw (~928).  TWO pairs form a GROUP sharing one
activation instruction (amortizes the ~500-cycle ACT fixed cost):
  * 4 matmuls fill a [128, 2*padw] PSUM tile (fp8 lhsT/rhs, 120
    contraction rows: 112 pairwise-g + 8 L rows via a 1/16 indicator
    block against L*16 for fp8 precision),
  * ONE exp on the Act engine over both pairs (FD=2*padw) with
    accum_out -> a PSUM column: den0+den1 of the group,
  * per pair a DVE scalar_tensor_tensor against the y-table with
    accum_out -> num columns,
  * a 4x-rate DVE tensor_scalar accum over pair 0 -> den0,
  * a tiny GPSIMD copy extracts the group-den column before PSUM reuse.
Host recovers den1 = group_den - den0, then finishes with the residual
add, query mask, and the c_in->c_out linear.  The whole pairwise_g
stream and the stationary are fp8-e4m3 (halves DMA; verified rel-l2
~4e-4 vs the f32 reference, gate is 2e-2).

Sharding: query dim i is split 8 ways (16 i x 4 b = 32 pairs per core).
"""

import numpy as np

import concourse.bacc as bacc
import concourse.tile as tile
from concourse import mybir
from concourse.bass_utils import run_bass_kernel_spmd

B, N, S, CIN, COUT, GDIM = 4, 128, 8, 8, 8, 7
NCORES = 8
ISHARD = N // NCORES          # 16 query points per core
NBLK = B * ISHARD             # 64 (b,i) blocks per core
NPAIR = NBLK // 2             # 32 block pairs per core
NGRP = NPAIR // 2             # 16 two-pair groups per core
CROWS = 120                   # contraction rows: 112 pairwise-g + 8 L
LSCALE = 16.0                 # L rows stored *16, indicator block 1/16

# key axis is COMPACTED per batch to the mask-valid columns, padded to a
# static width padw; _host_prep computes it from the mask.
_LAYOUT = {"padw": 928}

# groups covered by each stream DMA (earlier ones smaller for ramp)
SUPER_GRPS = (1, 2, 2, 3, 4, 4)

F32 = mybir.dt.float32
BF16 = mybir.dt.bfloat16
FP8 = mybir.dt.float8e4
NPBF16 = mybir.dt.np(mybir.dt.bfloat16)
NPF8 = mybir.dt.np(mybir.dt.float8e4)

_PROGRAM_CACHE = {}


def _build_program(nblk=NBLK, loop_reps=1, mode="full"):
    """loop_reps>1 wraps the main loop in a hardware For_i re-running the
    full pass (input DMAs included) for slope timing.

    mode: base[+flags]; bases: full, nostt (skip DVE/den), noexp
    (matmul+DMA only), dmaonly, nodma.  flags: sb (single buffer),
    e6/e8 (epool bufs), fine (finer DMA supers), na (drop exp accum,
    timing only), dved (den extract on DVE instead of GPSIMD), bden
    (no exp accum; both dens via DVE tensor_scalar)."""
    nc = bacc.Bacc("TRN2", target_bir_lowering=False, debug=False,
                   num_devices=NCORES)

    padw = _LAYOUT["padw"]
    gw = 2 * padw                 # group free width
    ngrp = nblk // 4
    npair = nblk // 2

    strm_d = nc.dram_tensor("strm", (CROWS, NPAIR * padw), FP8,
                            kind="ExternalInput").ap()
    lhsT_d = nc.dram_tensor("lhsT", (CROWS, 128), FP8,
                            kind="ExternalInput").ap()
    yt_d = nc.dram_tensor("yt", (128, B * padw), BF16,
                          kind="ExternalInput").ap()
    out_d = nc.dram_tensor("out_s", (128, 2 * NPAIR), F32,
                           kind="ExternalOutput").ap()

    if mode.startswith(("actb", "dveb", "mmb", "ttrb", "redb", "ttb")):
        return _build_microbench(nc, out_d, mode, loop_reps)

    parts = mode.split("+")
    mode = parts[0]
    flags = set(parts[1:])
    nbuf = 1 if (loop_reps == 1 or "sb" in flags) else 2
    ep_bufs = 8 if "e8" in flags else (6 if "e6" in flags else 4)
    sup_grps = SUPER_GRPS
    if "fine" in flags:
        sup_grps = (1, 1, 2, 2, 2, 2, 3, 3)
    use_accum = ("na" not in flags) and ("bden" not in flags)
    # GPSIMD cannot read PSUM, so the den-column extract rides on DVE
    extract_eng = "dve"
    both_den_dve = "bden" in flags
    # PSUM: 2 group tiles of 4 banks each fill all 8 banks; the exp accum
    # column lives in the tile's unused tail (needs gw < 2048)
    assert gw < 2048, "padw too large for in-tile accum column"
    supers = []
    g0 = 0
    for ng in sup_grps:
        if g0 >= ngrp:
            break
        ng = min(ng, ngrp - g0)
        supers.append((g0, g0 + ng))
        g0 += ng

    with tile.TileContext(nc) as tc:
        with (
            tc.tile_pool(name="consts", bufs=1) as consts,
            tc.tile_pool(name="epool", bufs=ep_bufs) as epool,
            tc.tile_pool(name="ps", bufs=2, space="PSUM") as ps,
        ):
            strm_bufs = [consts.tile([CROWS, NPAIR * padw], FP8,
                                     name=f"s{i}") for i in range(nbuf)]
            lhsT_bufs = [consts.tile([CROWS, 128], FP8, name=f"w{i}")
                         for i in range(nbuf)]
            yt_bufs = [consts.tile([128, B * padw], BF16, name=f"y{i}")
                       for i in range(nbuf)]
            res_bufs = [consts.tile([128, NPAIR], F32, name=f"r{i}")
                        for i in range(nbuf)]
            dg_bufs = [consts.tile([128, NGRP], F32, name=f"dg{i}")
                       for i in range(nbuf)]
            d0_bufs = [consts.tile([128, NGRP], F32, name=f"d0{i}")
                       for i in range(nbuf)]
            scr = consts.tile([128, padw], BF16)
            scr2 = consts.tile([128, padw], BF16)
            warm = consts.tile([128, 1], F32)

            # preload the exp table set before the main loop
            nc.vector.memset(warm, 0.0)
            nc.scalar.activation(warm, warm,
                                 mybir.ActivationFunctionType.Exp)
            if mode in ("dmaonly", "noexp", "nostt"):
                for r in res_bufs + dg_bufs + d0_bufs:
                    nc.vector.memset(r, 1.0)
            if mode == "nodma":
                for r in strm_bufs:
                    nc.vector.memset(r, 0.01)
                for r in lhsT_bufs:
                    nc.vector.memset(r, 0.01)
                for r in yt_bufs:
                    nc.vector.memset(r, 0.01)

            def main_pass(strm, lhsT, yt, res, dg, d0):
                if mode != "nodma":
                    nc.sync.dma_start(lhsT, lhsT_d)
                    nc.sync.dma_start(yt, yt_d)
                    for (q0, q1) in supers:
                        nc.sync.dma_start(
                            strm[:, q0 * gw:q1 * gw],
                            strm_d[:, q0 * gw:q1 * gw])
                for g in range(ngrp):
                    if mode == "dmaonly":
                        break
                    b = g // (ngrp // B)
                    col0 = g * gw
                    l_ps = ps.tile([128, 2048], F32, tag="l")
                    for hh in range(0, gw, 512):
                        he = min(hh + 512, gw)
                        nc.tensor.matmul(
                            l_ps[:, hh:he], lhsT=lhsT,
                            rhs=strm[:, col0 + hh:col0 + he],
                            start=True, stop=True)
                    if mode == "noexp":
                        continue
                    e_t = epool.tile([128, gw], BF16, tag="e")
                    nc.scalar.activation(
                        e_t, l_ps[:, 0:gw],
                        mybir.ActivationFunctionType.Exp,
                        accum_out=(l_ps[:, 2047:2048]
                                   if use_accum else None))
                    if mode == "nostt":
                        continue
                    # extract first: it releases the PSUM tile for the
                    # pipelined matmuls two groups ahead
                    if use_accum:
                        nc.vector.tensor_copy(dg[:, g:g + 1],
                                              l_ps[:, 2047:2048])
                    yt_b = yt[:, b * padw:(b + 1) * padw]
                    nc.vector.scalar_tensor_tensor(
                        scr, e_t[:, 0:padw], 0.0, yt_b,
                        op0=mybir.AluOpType.bypass,
                        op1=mybir.AluOpType.mult,
                        accum_out=res[:, 2 * g:2 * g + 1])
                    nc.vector.scalar_tensor_tensor(
                        scr, e_t[:, padw:gw], 0.0, yt_b,
                        op0=mybir.AluOpType.bypass,
                        op1=mybir.AluOpType.mult,
                        accum_out=res[:, 2 * g + 1:2 * g + 2])
                    nc.vector.tensor_scalar(
                        scr2, e_t[:, 0:padw], 1.0, 0.0,
                        op0=mybir.AluOpType.mult,
                        op1=mybir.AluOpType.add,
                        accum_out=d0[:, g:g + 1])
                    if both_den_dve:
                        nc.vector.tensor_scalar(
                            scr2, e_t[:, padw:gw], 1.0, 0.0,
                            op0=mybir.AluOpType.mult,
                            op1=mybir.AluOpType.add,
                            accum_out=dg[:, g:g + 1])
                    elif use_accum:
                        nc.vector.tensor_copy(dg[:, g:g + 1],
                                              l_ps[:, 2047:2048])
                if "noo" not in flags:
                    nc.sync.dma_start(out_d[:, 0:NGRP], dg)
                    nc.sync.dma_start(out_d[:, NGRP:2 * NGRP], d0)
                    nc.sync.dma_start(out_d[:, NPAIR:2 * NPAIR], res)

            if loop_reps > 1:
                if nbuf == 2:
                    assert loop_reps % 2 == 0, "loop_reps must be even"
                with tc.For_i(0, loop_reps // nbuf, 1,
                              hint_engines=(mybir.EngineType.PE,
                                            mybir.EngineType.Activation,
                                            mybir.EngineType.DVE,
                                            mybir.EngineType.Pool,
                                            mybir.EngineType.SP)):
                    for ib in range(nbuf):
                        main_pass(strm_bufs[ib], lhsT_bufs[ib],
                                  yt_bufs[ib], res_bufs[ib],
                                  dg_bufs[ib], d0_bufs[ib])
            else:
                main_pass(strm_bufs[0], lhsT_bufs[0], yt_bufs[0],
                          res_bufs[0], dg_bufs[0], d0_bufs[0])

    nc.compile()
    return nc


def _build_microbench(nc, out_d, mode, loop_reps):
    """Pure per-engine instruction pacing benches: NI dependency-free
    instructions per iteration on one engine (same-engine WAW only)."""
    NI = 128
    width = 2048 if "2048" in mode else (1024 if "1024" in mode else 512)
    accum = "na" not in mode
    edt = mybir.dt.float8e4 if "f8" in mode else BF16
    accum_psum = mode.endswith("p")
    with tile.TileContext(nc) as tc:
        with (
            tc.tile_pool(name="consts", bufs=1) as consts,
            tc.tile_pool(name="epool", bufs=4) as epool,
            tc.tile_pool(name="ps", bufs=2, space="PSUM") as ps,
            tc.tile_pool(name="psc", bufs=1, space="PSUM") as psc,
        ):
            g_all = consts.tile([128, 4096], BF16)
            den_buf = consts.tile([128, NBLK], F32)
            num_buf = consts.tile([128, NBLK], F32)
            scr = consts.tile([128, width], edt)
            e_src = consts.tile([128, width], edt)
            warm = consts.tile([128, 1], F32)
            nc.vector.memset(warm, 0.0)
            nc.scalar.activation(warm, warm,
                                 mybir.ActivationFunctionType.Exp)
            nc.vector.memset(g_all, 0.01)
            nc.vector.memset(e_src, 1.0)
            nc.vector.memset(den_buf, 1.0)
            nc.vector.memset(num_buf, 1.0)
            l_ps = psc.tile([128, width], F32)
            if accum_psum:
                den_ps = psc.tile([128, NBLK], F32)
            for h in range(0, width, 512):
                nc.tensor.matmul(l_ps[:, h:h + 512], lhsT=g_all[:, 0:128],
                                 rhs=g_all[:, 128 + h:640 + h],
                                 start=True, stop=True)

            def body():
                for k in range(NI):
                    if mode.startswith("actb"):
                        e_t = epool.tile([128, width], edt, tag="e")
                        tgt = den_ps if accum_psum else den_buf
                        nc.scalar.activation(
                            e_t, l_ps, mybir.ActivationFunctionType.Exp,
                            accum_out=(tgt[:, k % NBLK:k % NBLK + 1]
                                       if accum else None))
                    elif mode.startswith("dveb"):
                        if "ts" in mode:
                            nc.vector.tensor_scalar(
                                scr, e_src, 1.0, 0.0,
                                op0=mybir.AluOpType.mult,
                                op1=mybir.AluOpType.add,
                                accum_out=num_buf[:, k % NBLK:k % NBLK + 1])
                        else:
                            nc.vector.scalar_tensor_tensor(
                                scr, e_src, 0.0, g_all[:, 128:128 + width],
                                op0=mybir.AluOpType.bypass,
                                op1=mybir.AluOpType.mult,
                                accum_out=num_buf[:, k % NBLK:k % NBLK + 1])
                    elif mode.startswith("redb"):
                        nc.vector.tensor_reduce(
                            num_buf[:, k % NBLK:k % NBLK + 1], e_src,
                            axis=mybir.AxisListType.X,
                            op=mybir.AluOpType.add)
                    elif mode.startswith("ttb"):
                        nc.vector.tensor_tensor(
                            scr, e_src, g_all[:, 128:128 + width],
                            op=mybir.AluOpType.mult)
                    else:  # mmb
                        o = ps.tile([128, 512], F32, tag="l")
                        nc.tensor.matmul(o, lhsT=g_all[:, 0:128],
                                         rhs=g_all[:, 128:640],
                                         start=True, stop=True)
                nc.sync.dma_start(out_d[:, 0:NPAIR], den_buf[:, 0:NPAIR])
                nc.sync.dma_start(out_d[:, NPAIR:2 * NPAIR],
                                  num_buf[:, 0:NPAIR])

            if loop_reps > 1:
                with tc.For_i(0, loop_reps, 1,
                              hint_engines=(mybir.EngineType.PE,
                                            mybir.EngineType.Activation,
                                            mybir.EngineType.DVE,
                                            mybir.EngineType.SP)):
                    body()
            else:
                body()
    nc.compile()
    return nc


def _get_program(nblk=NBLK, loop_reps=1, mode="full"):
    key = ("nc", nblk, loop_reps, mode, _LAYOUT["padw"])
    if key not in _PROGRAM_CACHE:
        _PROGRAM_CACHE[key] = _build_program(nblk, loop_reps, mode)
    return _PROGRAM_CACHE[key]


def _host_prep(pairwise_g, coset_functions, mask, w_y, w_g):
    """Build the per-core fp8/bf16 input tensors with a mask-compacted
    key axis.

    The free (key) axis is the flat (s2, j) list of mask-valid columns per
    batch, padded to a static width padw; pad columns carry L = -240 so
    their exp is ~0 (<=2e-6 after the 1/16 indicator).  Order is
    irrelevant (den/num are plain sums)."""
    y = np.asarray(coset_functions, dtype=np.float32)    # (B, N, S, C)
    maskb = np.asarray(mask)
    w_y0 = np.asarray(w_y, dtype=np.float32)[:, 0]
    w_g = np.asarray(w_g, dtype=np.float32)

    # valid flat key indices v = s2*N + j, per batch
    mflat = maskb.transpose(0, 2, 1).reshape(B, S * N)
    counts = mflat.sum(axis=1)
    padw = max(32, int(-(-int(counts.max()) // 32) * 32))
    _LAYOUT.update(padw=padw)

    gidx = np.zeros((B, padw), np.int64)
    pad = np.ones((B, padw), bool)
    for b in range(B):
        ix = np.flatnonzero(mflat[b])
        gidx[b, :len(ix)] = ix
        pad[b, :len(ix)] = False

    # ycols[b, v, c] = y[b, j(v), s2(v), c]
    yv = y.transpose(0, 2, 1, 3).reshape(B, S * N, CIN)
    ycols = np.stack([yv[b, gidx[b]] for b in range(B)])  # (B, padw, C)
    # L rows (C, padw) *LSCALE: w_y0*y on valid cols, -240 on pads
    ld = LSCALE * (w_y0 * ycols)
    ld[pad] = -240.0
    ld = np.ascontiguousarray(ld.transpose(0, 2, 1))      # (B, C, padw)
    # y table, zeroed on pads, duplicated over (i2, s1)
    ytab = np.where(pad[..., None], 0.0, ycols).transpose(0, 2, 1)
    yt = np.broadcast_to(ytab[:, None], (B, 16, CIN, padw))
    yt_plane = yt.reshape(B, 128, padw).transpose(1, 0, 2).reshape(128, -1)
    yt_plane = np.ascontiguousarray(yt_plane, dtype=NPBF16)

    # stationary lhsT (120, 128): out col k = i2*64 + s1*8 + c
    lhsT = np.zeros((CROWS, 128), np.float32)
    for i2 in range(2):
        for s1 in range(S):
            for g in range(GDIM):
                p = i2 * 56 + s1 * GDIM + g
                k0 = i2 * 64 + s1 * 8
                lhsT[p, k0:k0 + CIN] = w_g[:, g]
    for c in range(CIN):
        for i2 in range(2):
            for s1 in range(S):
                lhsT[112 + c, i2 * 64 + s1 * 8 + c] = 1.0 / LSCALE
    lhsT = np.ascontiguousarray(lhsT, dtype=NPF8)

    pairwise_g = np.asarray(pairwise_g, dtype=np.float32)
    in_maps = []
    for k in range(NCORES):
        sl = slice(ISHARD * k, ISHARD * (k + 1))
        pg = pairwise_g[:, sl]                   # (B, 16, j, s1, s2, g)
        x = pg.reshape(B, 8, 2, N, S, S, GDIM)
        x = x.transpose(0, 1, 2, 4, 6, 5, 3)     # [b,ip,i2,s1,g,s2,j]
        x = np.ascontiguousarray(x).reshape(B, 8, 2, S, GDIM, S * N)
        xg = np.stack([x[b][..., gidx[b]] for b in range(B)])
        xg = xg.reshape(NPAIR, 112, padw)
        strm = np.empty((CROWS, NPAIR, padw), NPF8)
        strm[0:112] = xg.transpose(1, 0, 2)
        strm[112:CROWS] = np.repeat(ld, NPAIR // B, axis=0) \
            .transpose(1, 0, 2)
        in_maps.append({"strm": strm.reshape(CROWS, -1),
                        "lhsT": lhsT, "yt": yt_plane})
    return in_maps


def _host_finish(s_list, coset_functions, mask, w_lin):
    """Decode per-core (128, 64) outputs into the full result."""
    y = np.asarray(coset_functions, dtype=np.float32)
    maskf = np.asarray(mask).astype(np.float32)
    w_lin = np.asarray(w_lin, dtype=np.float32)
    out = np.empty((B, N, S, COUT), np.float32)
    for k in range(NCORES):
        s = np.asarray(s_list[k], dtype=np.float32)      # (128, 64)
        dgrp = s[:, 0:NGRP]
        d0 = s[:, NGRP:2 * NGRP]
        num = s[:, NPAIR:]
        den = np.empty((128, NPAIR), np.float32)
        den[:, 0::2] = d0
        den[:, 1::2] = dgrp - d0
        # partition p = i2*64 + s1*8 + c; block = 2*pp + i2
        den = den.reshape(2, S, CIN, NPAIR).transpose(3, 0, 1, 2)
        num = num.reshape(2, S, CIN, NPAIR).transpose(3, 0, 1, 2)
        den = den.reshape(NBLK, S, CIN)
        num = num.reshape(NBLK, S, CIN)
        sl = slice(ISHARD * k, ISHARD * (k + 1))
        y_q = y[:, sl].reshape(NBLK, S, CIN)
        m_q = maskf[:, sl].reshape(NBLK, S)
        res = (y_q + num / den) * m_q[..., None]
        res = res @ w_lin.T
        out[:, sl] = res.reshape(B, ISHARD, S, COUT)
    return out


def kernel(pairwise_g, coset_functions, mask, w_y, b_y, w_g, b_g, w_lin):
    pairwise_g = np.asarray(pairwise_g)
    coset_functions = np.asarray(coset_functions)
    mask = np.asarray(mask)

    in_maps = _host_prep(pairwise_g, coset_functions, mask,
                         np.asarray(w_y), np.asarray(w_g))
    nc = _get_program()
    res = run_bass_kernel_spmd(nc, in_maps, core_ids=list(range(NCORES)))
    s_list = [r["out_s"] for r in res.results]
    return _host_finish(s_list, coset_functions, mask, np.asarray(w_lin))


# revision 24
# speedup vs baseline: 1.4524x; 1.4524x over previous
"""Trainium2 Bass kernel for equivariant multihead attention.

Math (per batch b, query point i, coset s1, channel c):
    logit[j,s2] = sum_g pairwise_g[b,i,j,s1,s2,g]*w_g[c,g]
                  + w_y[c,0]*y[b,j,s2,c] + w_y[c,1]*y[b,i,s1,c] + b_g[c] + b_y[c]
    att = exp(logit)*mask[b,j,s2];  att /= sum_{j,s2} att
    out = (y[b,i,s1,c] + sum_{j,s2} att*y[b,j,s2,c]) * mask[b,i,s1]  @ w_lin.T

The query-side term and the biases are constant over the key dims (j,s2) and
cancel in the normalization, so they are dropped.  The key-side factor
exp(w_y[c,0]*y)*mask is folded INTO the exponent: with
    L[(s2,c), j] = w_y0[c]*y[b,j,s2,c] + log(mask[b,j,s2])   (pad -> -240)
the unnormalized attention is E' = exp(sum_g pg*w_g + L) directly.

TWO query points (a "pair") are packed per output tile: partition is
(i2, s1, c) = 2*8*8 = 128; the free dim is the mask-compacted key list
(s2, j), padded to padw (~928).  TWO pairs form a GROUP sharing one
activation instruction (amortizes the ~500-cycle ACT fixed cost):
  * 4 matmuls fill a [128, 2*padw] PSUM tile (fp8 lhsT/rhs, 120
    contraction rows: 112 pairwise-g + 8 L rows via a 1/16 indicator
    block against L*16 for fp8 precision),
  * ONE exp on the Act engine over both pairs (FD=2*padw) with
    accum_out -> a PSUM column: den0+den1 of the group,
  * per pair a DVE scalar_tensor_tensor against the y-table with
    accum_out -> num columns,
  * a 4x-rate DVE tensor_scalar accum over pair 0 -> den0,
  * a tiny GPSIMD copy extracts the group-den column before PSUM reuse.
Host recovers den1 = group_den - den0, then finishes with the residual
add, query mask, and the c_in->c_out linear.  The whole pairwise_g
stream and the stationary are fp8-e4m3 (halves DMA; verified rel-l2
~4e-4 vs the f32 reference, gate is 2e-2).

Sharding: query dim i is split 8 ways (16 i x 4 b = 32 pairs per core).
"""

import numpy as np

import concourse.bacc as bacc
import concourse.tile as tile
from concourse import mybir
from concourse.bass_utils import run_bass_kernel_spmd

B, N, S, CIN, COUT, GDIM = 4, 128, 8, 8, 8, 7
NCORES = 8
ISHARD = N // NCORES          # 16 query points per core
NBLK = B * ISHARD             # 64 (b,i) blocks per core
NPAIR = NBLK // 2             # 32 block pairs per core
NGRP = NPAIR // 2             # 16 two-pair groups per core
CROWS = 120                   # contraction rows: 112 pairwise-g + 8 L
LSCALE = 16.0                 # L rows stored *16, indicator block 1/16

# key axis is COMPACTED per batch to the mask-valid columns, padded to a
# static width padw; _host_prep computes it from the mask.
_LAYOUT = {"padw": 928}

# groups covered by each stream DMA (earlier ones smaller for ramp)
SUPER_GRPS = (1, 2, 2, 3, 4, 4)

# program variant used by kernel(): per-pair exps ("full+sp", den via the
# exp accum in the PSUM tile tail) with the custom affine_mul_reduce num
DEFAULT_MODE = "full+sp"

F32 = mybir.dt.float32
BF16 = mybir.dt.bfloat16
FP8 = mybir.dt.float8e4
NPBF16 = mybir.dt.np(mybir.dt.bfloat16)
NPF8 = mybir.dt.np(mybir.dt.float8e4)

_PROGRAM_CACHE = {}


def _build_program(nblk=NBLK, loop_reps=1, mode="full"):
    """loop_reps>1 wraps the main loop in a hardware For_i re-running the
    full pass (input DMAs included) for slope timing.

    mode: base[+flags]; bases: full, nostt (skip DVE/den), noexp
    (matmul+DMA only), dmaonly, nodma.  flags: sb (single buffer),
    e6/e8 (epool bufs), fine (finer DMA supers), na (drop exp accum,
    timing only), dved (den extract on DVE instead of GPSIMD), bden
    (no exp accum; both dens via DVE tensor_scalar)."""
    nc = bacc.Bacc("TRN2", target_bir_lowering=False, debug=False,
                   num_devices=NCORES)

    padw = _LAYOUT["padw"]
    gw = 2 * padw                 # group free width
    ngrp = nblk // 4
    npair = nblk // 2

    strm_d = nc.dram_tensor("strm", (CROWS, NPAIR * padw), FP8,
                            kind="ExternalInput").ap()
    lhsT_d = nc.dram_tensor("lhsT", (CROWS, 128), FP8,
                            kind="ExternalInput").ap()
    yt_d = nc.dram_tensor("yt", (128, B * padw), BF16,
                          kind="ExternalInput").ap()
    out_d = nc.dram_tensor("out_s", (128, 2 * NPAIR), F32,
                           kind="ExternalOutput").ap()

    if mode.startswith(("actb", "dveb", "mmb", "ttrb", "redb", "ttb",
                        "amrb", "poolts", "pooltt", "poolb")):
        return _build_microbench(nc, out_d, mode, loop_reps)

    parts = mode.split("+")
    mode = parts[0]
    flags = set(parts[1:])
    nbuf = 1 if (loop_reps == 1 or "sb" in flags) else 2
    ep_bufs = 8 if "e8" in flags else (6 if "e6" in flags else 4)
    sup_grps = SUPER_GRPS
    if "fine" in flags:
        sup_grps = (1, 1, 2, 2, 2, 2, 3, 3)
    use_accum = "na" not in flags
    split = "sp" in flags        # one exp per pair instead of per group
    stock_num = "sttn" in flags  # stock stt for num instead of custom amr
    stock_den = "tsd" in flags   # stock ts for den0 instead of amr+ones
    sbuf_den = "sbd" in flags    # split: exp accum straight to SBUF dg
    # PSUM: merged = 2 group tiles of 4 banks; split = 4 pair tiles of
    # 2 banks.  The exp accum column lives in the tile's unused tail.
    assert gw < 2048, "padw too large for in-tile accum column"
    supers = []
    g0 = 0
    for ng in sup_grps:
        if g0 >= ngrp:
            break
        ng = min(ng, ngrp - g0)
        supers.append((g0, g0 + ng))
        g0 += ng

    with tile.TileContext(nc) as tc:
        with (
            tc.tile_pool(name="consts", bufs=1) as consts,
            tc.tile_pool(name="epool", bufs=ep_bufs) as epool,
            tc.tile_pool(name="ps", bufs=(4 if split else 2),
                         space="PSUM") as ps,
        ):
            strm_bufs = [consts.tile([CROWS, NPAIR * padw], FP8,
                                     name=f"s{i}") for i in range(nbuf)]
            lhsT_bufs = [consts.tile([CROWS, 128], FP8, name=f"w{i}")
                         for i in range(nbuf)]
            yt_bufs = [consts.tile([128, B * padw], BF16, name=f"y{i}")
                       for i in range(nbuf)]
            res_bufs = [consts.tile([128, NPAIR], F32, name=f"r{i}")
                        for i in range(nbuf)]
            dg_bufs = [consts.tile([128, NPAIR], F32, name=f"dg{i}")
                       for i in range(nbuf)]
            d0_bufs = [consts.tile([128, NGRP], F32, name=f"d0{i}")
                       for i in range(nbuf)]
            scr = consts.tile([128, padw], BF16)
            scr2 = consts.tile([128, padw], BF16)
            ones = consts.tile([128, padw], BF16)
            warm = consts.tile([128, 1], F32)

            # preload the exp table set before the main loop
            nc.vector.memset(warm, 0.0)
            nc.scalar.activation(warm, warm,
                                 mybir.ActivationFunctionType.Exp)
            nc.vector.memset(ones, 1.0)
            if mode in ("dmaonly", "noexp", "nostt"):
                for r in res_bufs + dg_bufs + d0_bufs:
                    nc.vector.memset(r, 1.0)
            if mode == "nodma":
                for r in strm_bufs:
                    nc.vector.memset(r, 0.01)
                for r in lhsT_bufs:
                    nc.vector.memset(r, 0.01)
                for r in yt_bufs:
                    nc.vector.memset(r, 0.01)

            def num_op(e_sl, yt_b, acc):
                if stock_num:
                    nc.vector.scalar_tensor_tensor(
                        scr, e_sl, 0.0, yt_b,
                        op0=mybir.AluOpType.bypass,
                        op1=mybir.AluOpType.mult, accum_out=acc)
                else:
                    nc.vector.affine_mul_reduce(
                        out=scr, accum_out=acc, in0=e_sl, in1=yt_b,
                        scale=1.0, bias=0.0)

            def den_op(e_sl, acc):
                if stock_den:
                    nc.vector.tensor_scalar(
                        scr2, e_sl, 1.0, 0.0,
                        op0=mybir.AluOpType.mult,
                        op1=mybir.AluOpType.add, accum_out=acc)
                else:
                    nc.vector.affine_mul_reduce(
                        out=scr2, accum_out=acc, in0=e_sl, in1=ones,
                        scale=1.0, bias=0.0)

            def merged_group(strm, lhsT, yt, res, dg, d0, g):
                b = g // (ngrp // B)
                col0 = g * gw
                l_ps = ps.tile([128, 2048], F32, tag="l")
                for hh in range(0, gw, 512):
                    he = min(hh + 512, gw)
                    nc.tensor.matmul(
                        l_ps[:, hh:he], lhsT=lhsT,
                        rhs=strm[:, col0 + hh:col0 + he],
                        start=True, stop=True)
                if mode == "noexp":
                    return
                e_t = epool.tile([128, gw], BF16, tag="e")
                nc.scalar.activation(
                    e_t, l_ps[:, 0:gw],
                    mybir.ActivationFunctionType.Exp,
                    accum_out=(l_ps[:, 2047:2048] if use_accum else None))
                if mode == "nostt":
                    return
                # extract first: it releases the PSUM tile for the
                # pipelined matmuls two groups ahead
                if use_accum:
                    nc.vector.tensor_copy(dg[:, g:g + 1],
                                          l_ps[:, 2047:2048])
                yt_b = yt[:, b * padw:(b + 1) * padw]
                num_op(e_t[:, 0:padw], yt_b, res[:, 2 * g:2 * g + 1])
                num_op(e_t[:, padw:gw], yt_b, res[:, 2 * g + 1:2 * g + 2])
                den_op(e_t[:, 0:padw], d0[:, g:g + 1])

            def split_pair(strm, lhsT, yt, res, dg, pp):
                b = pp // (npair // B)
                col0 = pp * padw
                l_ps = ps.tile([128, 1024], F32, tag="l")
                for hh in range(0, padw, 512):
                    he = min(hh + 512, padw)
                    nc.tensor.matmul(
                        l_ps[:, hh:he], lhsT=lhsT,
                        rhs=strm[:, col0 + hh:col0 + he],
                        start=True, stop=True)
                if mode == "noexp":
                    return
                e_t = epool.tile([128, padw], BF16, tag="e")
                acc = None
                if use_accum:
                    acc = dg[:, pp:pp + 1] if sbuf_den else l_ps[:, 1023:1024]
                nc.scalar.activation(
                    e_t, l_ps[:, 0:padw],
                    mybir.ActivationFunctionType.Exp,
                    accum_out=acc)
                if mode == "nostt":
                    return
                if use_accum and not sbuf_den:
                    nc.vector.tensor_copy(dg[:, pp:pp + 1],
                                          l_ps[:, 1023:1024])
                yt_b = yt[:, b * padw:(b + 1) * padw]
                num_op(e_t, yt_b, res[:, pp:pp + 1])

            def main_pass(strm, lhsT, yt, res, dg, d0):
                if mode != "nodma":
                    nc.sync.dma_start(lhsT, lhsT_d)
                    nc.sync.dma_start(yt, yt_d)
                    for (q0, q1) in supers:
                        nc.sync.dma_start(
                            strm[:, q0 * gw:q1 * gw],
                            strm_d[:, q0 * gw:q1 * gw])
                if mode != "dmaonly":
                    if split:
                        for pp in range(npair):
                            split_pair(strm, lhsT, yt, res, dg, pp)
                    else:
                        for g in range(ngrp):
                            merged_group(strm, lhsT, yt, res, dg, d0, g)
                if "noo" not in flags:
                    # layout: merged [dgrp 16 | d0 16 | num 32],
                    #         split  [den 32 | num 32]
                    if split:
                        nc.sync.dma_start(out_d[:, 0:NPAIR],
                                          dg[:, 0:NPAIR])
                    else:
                        nc.sync.dma_start(out_d[:, 0:NGRP],
                                          dg[:, 0:NGRP])
                        nc.sync.dma_start(out_d[:, NGRP:2 * NGRP], d0)
                    nc.sync.dma_start(out_d[:, NPAIR:2 * NPAIR], res)

            if loop_reps > 1:
                if nbuf == 2:
                    assert loop_reps % 2 == 0, "loop_reps must be even"
                with tc.For_i(0, loop_reps // nbuf, 1,
                              hint_engines=(mybir.EngineType.PE,
                                            mybir.EngineType.Activation,
                                            mybir.EngineType.DVE,
                                            mybir.EngineType.Pool,
                                            mybir.EngineType.SP)):
                    for ib in range(nbuf):
                        main_pass(strm_bufs[ib], lhsT_bufs[ib],
                                  yt_bufs[ib], res_bufs[ib],
                                  dg_bufs[ib], d0_bufs[ib])
            else:
                main_pass(strm_bufs[0], lhsT_bufs[0], yt_bufs[0],
                          res_bufs[0], dg_bufs[0], d0_bufs[0])

    nc.compile()
    return nc


def _build_microbench(nc, out_d, mode, loop_reps):
    """Pure per-engine instruction pacing benches: NI dependency-free
    instructions per iteration on one engine (same-engine WAW only)."""
    NI = 128
    width = 2048 if "2048" in mode else (1024 if "1024" in mode else 512)
    accum = "na" not in mode
    edt = mybir.dt.float8e4 if "f8" in mode else BF16
    accum_psum = mode.endswith("p")
    with tile.TileContext(nc) as tc:
        with (
            tc.tile_pool(name="consts", bufs=1) as consts,
            tc.tile_pool(name="epool", bufs=4) as epool,
            tc.tile_pool(name="ps", bufs=2, space="PSUM") as ps,
            tc.tile_pool(name="psc", bufs=1, space="PSUM") as psc,
        ):
            g_all = consts.tile([128, 4096], BF16)
            den_buf = consts.tile([128, NBLK], F32)
            num_buf = consts.tile([128, NBLK], F32)
            scr = consts.tile([128, width], edt)
            e_src = consts.tile([128, width], edt)
            warm = consts.tile([128, 1], F32)
            nc.vector.memset(warm, 0.0)
            nc.scalar.activation(warm, warm,
                                 mybir.ActivationFunctionType.Exp)
            nc.vector.memset(g_all, 0.01)
            nc.vector.memset(e_src, 1.0)
            nc.vector.memset(den_buf, 1.0)
            nc.vector.memset(num_buf, 1.0)
            l_ps = psc.tile([128, width], F32)
            if accum_psum:
                den_ps = psc.tile([128, NBLK], F32)
            for h in range(0, width, 512):
                nc.tensor.matmul(l_ps[:, h:h + 512], lhsT=g_all[:, 0:128],
                                 rhs=g_all[:, 128 + h:640 + h],
                                 start=True, stop=True)

            def body():
                for k in range(NI):
                    if mode.startswith("actb"):
                        e_t = epool.tile([128, width], edt, tag="e")
                        tgt = den_ps if accum_psum else den_buf
                        nc.scalar.activation(
                            e_t, l_ps, mybir.ActivationFunctionType.Exp,
                            accum_out=(tgt[:, k % NBLK:k % NBLK + 1]
                                       if accum else None))
                    elif mode.startswith("dveb"):
                        if "ts" in mode:
                            nc.vector.tensor_scalar(
                                scr, e_src, 1.0, 0.0,
                                op0=mybir.AluOpType.mult,
                                op1=mybir.AluOpType.add,
                                accum_out=num_buf[:, k % NBLK:k % NBLK + 1])
                        else:
                            nc.vector.scalar_tensor_tensor(
                                scr, e_src, 0.0, g_all[:, 128:128 + width],
                                op0=mybir.AluOpType.bypass,
                                op1=mybir.AluOpType.mult,
                                accum_out=num_buf[:, k % NBLK:k % NBLK + 1])
                    elif mode.startswith("redb"):
                        nc.vector.tensor_reduce(
                            num_buf[:, k % NBLK:k % NBLK + 1], e_src,
                            axis=mybir.AxisListType.X,
                            op=mybir.AluOpType.add)
                    elif mode.startswith("ttrb"):
                        nc.vector.tensor_tensor_reduce(
                            scr, e_src, g_all[:, 128:128 + width],
                            1.0, 0.0,
                            op0=mybir.AluOpType.mult,
                            op1=mybir.AluOpType.add,
                            accum_out=num_buf[:, k % NBLK:k % NBLK + 1])
                    elif mode.startswith("ttb"):
                        nc.vector.tensor_tensor(
                            scr, e_src, g_all[:, 128:128 + width],
                            op=mybir.AluOpType.mult)
                    elif mode.startswith("amrb"):
                        nc.vector.affine_mul_reduce(
                            out=scr, in0=e_src,
                            in1=g_all[:, 128:128 + width],
                            scale=1.0, bias=0.0,
                            accum_out=num_buf[:, k % NBLK:k % NBLK + 1])
                    elif mode.startswith("poolts"):
                        nc.gpsimd.tensor_scalar(
                            scr, e_src, 1.0, 0.0,
                            op0=mybir.AluOpType.mult,
                            op1=mybir.AluOpType.add,
                            accum_out=num_buf[:, k % NBLK:k % NBLK + 1])
                    elif mode.startswith("pooltt"):
                        nc.gpsimd.tensor_tensor(
                            scr, e_src, g_all[:, 128:128 + width],
                            op=mybir.AluOpType.mult)
                    elif mode.startswith("poolb"):
                        nc.gpsimd.scalar_tensor_tensor(
                            scr, e_src, 0.0, g_all[:, 128:128 + width],
                            op0=mybir.AluOpType.bypass,
                            op1=mybir.AluOpType.mult,
                            accum_out=num_buf[:, k % NBLK:k % NBLK + 1])
                    else:  # mmb
                        o = ps.tile([128, 512], F32, tag="l")
                        nc.tensor.matmul(o, lhsT=g_all[:, 0:128],
                                         rhs=g_all[:, 128:640],
                                         start=True, stop=True)
                nc.sync.dma_start(out_d[:, 0:NPAIR], den_buf[:, 0:NPAIR])
                nc.sync.dma_start(out_d[:, NPAIR:2 * NPAIR],
                                  num_buf[:, 0:NPAIR])

            if loop_reps > 1:
                with tc.For_i(0, loop_reps, 1,
                              hint_engines=(mybir.EngineType.PE,
                                            mybir.EngineType.Activation,
                                            mybir.EngineType.DVE,
                                            mybir.EngineType.SP)):
                    body()
            else:
                body()
    nc.compile()
    return nc


def _get_program(nblk=NBLK, loop_reps=1, mode="full"):
    key = ("nc", nblk, loop_reps, mode, _LAYOUT["padw"])
    if key not in _PROGRAM_CACHE:
        _PROGRAM_CACHE[key] = _build_program(nblk, loop_reps, mode)
    return _PROGRAM_CACHE[key]


def _host_prep(pairwise_g, coset_functions, mask, w_y, w_g):
    """Build the per-core fp8/bf16 input tensors with a mask-compacted
    key axis.

    The free (key) axis is the flat (s2, j) list of mask-valid columns per
    batch, padded to a static width padw; pad columns carry L = -240 so
    their exp is ~0 (<=2e-6 after the 1/16 indicator).  Order is
    irrelevant (den/num are plain sums)."""
    y = np.asarray(coset_functions, dtype=np.float32)    # (B, N, S, C)
    maskb = np.asarray(mask)
    w_y0 = np.asarray(w_y, dtype=np.float32)[:, 0]
    w_g = np.asarray(w_g, dtype=np.float32)

    # valid flat key indices v = s2*N + j, per batch
    mflat = maskb.transpose(0, 2, 1).reshape(B, S * N)
    counts = mflat.sum(axis=1)
    padw = max(32, int(-(-int(counts.max()) // 32) * 32))
    _LAYOUT.update(padw=padw)

    gidx = np.zeros((B, padw), np.int64)
    pad = np.ones((B, padw), bool)
    for b in range(B):
        ix = np.flatnonzero(mflat[b])
        gidx[b, :len(ix)] = ix
        pad[b, :len(ix)] = False

    # ycols[b, v, c] = y[b, j(v), s2(v), c]
    yv = y.transpose(0, 2, 1, 3).reshape(B, S * N, CIN)
    ycols = np.stack([yv[b, gidx[b]] for b in range(B)])  # (B, padw, C)
    # L rows (C, padw) *LSCALE: w_y0*y on valid cols, -240 on pads
    ld = LSCALE * (w_y0 * ycols)
    ld[pad] = -240.0
    ld = np.ascontiguousarray(ld.transpose(0, 2, 1))      # (B, C, padw)
    # y table, zeroed on pads, duplicated over (i2, s1)
    ytab = np.where(pad[..., None], 0.0, ycols).transpose(0, 2, 1)
    yt = np.broadcast_to(ytab[:, None], (B, 16, CIN, padw))
    yt_plane = yt.reshape(B, 128, padw).transpose(1, 0, 2).reshape(128, -1)
    yt_plane = np.ascontiguousarray(yt_plane, dtype=NPBF16)

    # stationary lhsT (120, 128): out col k = i2*64 + s1*8 + c
    lhsT = np.zeros((CROWS, 128), np.float32)
    for i2 in range(2):
        for s1 in range(S):
            for g in range(GDIM):
                p = i2 * 56 + s1 * GDIM + g
                k0 = i2 * 64 + s1 * 8
                lhsT[p, k0:k0 + CIN] = w_g[:, g]
    for c in range(CIN):
        for i2 in range(2):
            for s1 in range(S):
                lhsT[112 + c, i2 * 64 + s1 * 8 + c] = 1.0 / LSCALE
    lhsT = np.ascontiguousarray(lhsT, dtype=NPF8)

    pairwise_g = np.asarray(pairwise_g, dtype=np.float32)
    in_maps = []
    for k in range(NCORES):
        sl = slice(ISHARD * k, ISHARD * (k + 1))
        pg = pairwise_g[:, sl]                   # (B, 16, j, s1, s2, g)
        x = pg.reshape(B, 8, 2, N, S, S, GDIM)
        x = x.transpose(0, 1, 2, 4, 6, 5, 3)     # [b,ip,i2,s1,g,s2,j]
        x = np.ascontiguousarray(x).reshape(B, 8, 2, S, GDIM, S * N)
        xg = np.stack([x[b][..., gidx[b]] for b in range(B)])
        xg = xg.reshape(NPAIR, 112, padw)
        strm = np.empty((CROWS, NPAIR, padw), NPF8)
        strm[0:112] = xg.transpose(1, 0, 2)
        strm[112:CROWS] = np.repeat(ld, NPAIR // B, axis=0) \
            .transpose(1, 0, 2)
        in_maps.append({"strm": strm.reshape(CROWS, -1),
                        "lhsT": lhsT, "yt": yt_plane})
    return in_maps


def _host_finish(s_list, coset_functions, mask, w_lin,
                 split=("sp" in DEFAULT_MODE)):
    """Decode per-core (128, 64) outputs into the full result."""
    y = np.asarray(coset_functions, dtype=np.float32)
    maskf = np.asarray(mask).astype(np.float32)
    w_lin = np.asarray(w_lin, dtype=np.float32)
    out = np.empty((B, N, S, COUT), np.float32)
    for k in range(NCORES):
        s = np.asarray(s_list[k], dtype=np.float32)      # (128, 64)
        num = s[:, NPAIR:]
        if split:
            den = s[:, 0:NPAIR].copy()
        else:
            dgrp = s[:, 0:NGRP]
            d0 = s[:, NGRP:2 * NGRP]
            den = np.empty((128, NPAIR), np.float32)
            den[:, 0::2] = d0
            den[:, 1::2] = dgrp - d0
        # partition p = i2*64 + s1*8 + c; block = 2*pp + i2
        den = den.reshape(2, S, CIN, NPAIR).transpose(3, 0, 1, 2)
        num = num.reshape(2, S, CIN, NPAIR).transpose(3, 0, 1, 2)
        den = den.reshape(NBLK, S, CIN)
        num = num.reshape(NBLK, S, CIN)
        sl = slice(ISHARD * k, ISHARD * (k + 1))
        y_q = y[:, sl].reshape(NBLK, S, CIN)
        m_q = maskf[:, sl].reshape(NBLK, S)
        res = (y_q + num / den) * m_q[..., None]
        res = res @ w_lin.T
        out[:, sl] = res.reshape(B, ISHARD, S, COUT)
    return out


def kernel(pairwise_g, coset_functions, mask, w_y, b_y, w_g, b_g, w_lin):
    pairwise_g = np.asarray(pairwise_g)
    coset_functions = np.asarray(coset_functions)
    mask = np.asarray(mask)

    in_maps = _host_prep(pairwise_g, coset_functions, mask,
                         np.asarray(w_y), np.asarray(w_g))
    nc = _get_program(NBLK, 1, DEFAULT_MODE)
    res = run_bass_kernel_spmd(nc, in_maps, core_ids=list(range(NCORES)))
    s_list = [r["out_s"] for r in res.results]
    return _host_finish(s_list, coset_functions, mask, np.asarray(w_lin))


# revision 28
# speedup vs baseline: 1.4642x; 1.0081x over previous
"""Trainium2 Bass kernel for equivariant multihead attention.

Math (per batch b, query point i, coset s1, channel c):
    logit[j,s2] = sum_g pairwise_g[b,i,j,s1,s2,g]*w_g[c,g]
                  + w_y[c,0]*y[b,j,s2,c] + w_y[c,1]*y[b,i,s1,c] + b_g[c] + b_y[c]
    att = exp(logit)*mask[b,j,s2];  att /= sum_{j,s2} att
    out = (y[b,i,s1,c] + sum_{j,s2} att*y[b,j,s2,c]) * mask[b,i,s1]  @ w_lin.T

The query-side term and the biases are constant over the key dims (j,s2) and
cancel in the normalization, so they are dropped.  The key-side factor
exp(w_y[c,0]*y)*mask is folded INTO the exponent: with
    L[(s2,c), j] = w_y0[c]*y[b,j,s2,c] + log(mask[b,j,s2])   (pad -> -240)
the unnormalized attention is E' = exp(sum_g pg*w_g + L) directly.

TWO query points (a "pair") are packed per output tile: partition is
(i2, s1, c) = 2*8*8 = 128; the free dim is the mask-compacted key list
(s2, j), padded to padw (~928).  Per pair (default "full+sp" mode):
  * 2 matmuls fill a [128, 1024] PSUM tile (fp8 lhsT/rhs, 120
    contraction rows: 112 pairwise-g + 8 L rows via a 1/16 indicator
    block against L*16 for fp8 precision),
  * one exp on the Act engine (FD=padw) with accum_out into the PSUM
    tile's tail column -> den (free; same-tile accum is the fast path),
  * a tiny DVE copy extracts the den column before the tile recycles,
  * one custom-DVE affine_mul_reduce against the y-table -> num.
The merged two-pair-per-exp variant ("full") trades ACT fixed cost for
an extra DVE reduction per group and measures slower overall (DVE is
within ~10% of ACT; reductions with accum_out run at ~1.2-1.5
cyc/elem on DVE regardless of op).  The whole pairwise_g stream and
the stationary are fp8-e4m3 (halves DMA; measured rel-l2 ~3.7e-4 vs
the f32 reference, gate is 2e-2).

Sharding: query dim i is split 8 ways (16 i x 4 b = 32 pairs per core).
"""

import numpy as np

import concourse.bacc as bacc
import concourse.tile as tile
from concourse import mybir
from concourse.bass_utils import run_bass_kernel_spmd

B, N, S, CIN, COUT, GDIM = 4, 128, 8, 8, 8, 7
NCORES = 8
ISHARD = N // NCORES          # 16 query points per core
NBLK = B * ISHARD             # 64 (b,i) blocks per core
NPAIR = NBLK // 2             # 32 block pairs per core
NGRP = NPAIR // 2             # 16 two-pair groups per core
CROWS = 120                   # contraction rows: 112 pairwise-g + 8 L
LSCALE = 16.0                 # L rows stored *16, indicator block 1/16

# key axis is COMPACTED per batch to the mask-valid columns, padded to a
# static width padw; _host_prep computes it from the mask.
_LAYOUT = {"padw": 928}

# groups covered by each stream DMA (earlier ones smaller for ramp)
SUPER_GRPS = (1, 2, 2, 3, 4, 4)

# program variant used by kernel(): per-pair exps ("full+sp", den via the
# exp accum in the PSUM tile tail) with the custom affine_mul_reduce num
DEFAULT_MODE = "full+sp"

F32 = mybir.dt.float32
BF16 = mybir.dt.bfloat16
FP8 = mybir.dt.float8e4
NPBF16 = mybir.dt.np(mybir.dt.bfloat16)
NPF8 = mybir.dt.np(mybir.dt.float8e4)

_PROGRAM_CACHE = {}


def _build_program(nblk=NBLK, loop_reps=1, mode="full"):
    """loop_reps>1 wraps the main loop in a hardware For_i re-running the
    full pass (input DMAs included) for slope timing.

    mode: base[+flags]; bases: full, nostt (skip DVE/den), noexp
    (matmul+DMA only), dmaonly, nodma.  flags: sp (per-pair exps; the
    production DEFAULT_MODE is "full+sp"), sb (single buffer), e6/e8
    (epool bufs), fine (finer DMA supers), na (drop exp accum, timing
    only), sttn/tsd (stock DVE ops instead of the custom
    affine_mul_reduce), sbd (exp accum to SBUF instead of the PSUM tile
    tail; measured slower), acp (every 4th den extract on ACT; measured
    slower)."""
    nc = bacc.Bacc("TRN2", target_bir_lowering=False, debug=False,
                   num_devices=NCORES)

    padw = _LAYOUT["padw"]
    gw = 2 * padw                 # group free width
    ngrp = nblk // 4
    npair = nblk // 2

    strm_d = nc.dram_tensor("strm", (CROWS, NPAIR * padw), FP8,
                            kind="ExternalInput").ap()
    lhsT_d = nc.dram_tensor("lhsT", (CROWS, 128), FP8,
                            kind="ExternalInput").ap()
    yt_d = nc.dram_tensor("yt", (128, B * padw), BF16,
                          kind="ExternalInput").ap()
    out_d = nc.dram_tensor("out_s", (128, 2 * NPAIR), F32,
                           kind="ExternalOutput").ap()

    if mode.startswith(("actb", "dveb", "mmb", "ttrb", "redb", "ttb",
                        "amrb", "poolts", "pooltt", "poolb")):
        return _build_microbench(nc, out_d, mode, loop_reps)

    parts = mode.split("+")
    mode = parts[0]
    flags = set(parts[1:])
    nbuf = 1 if (loop_reps == 1 or "sb" in flags) else 2
    ep_bufs = 8 if "e8" in flags else (6 if "e6" in flags else 4)
    sup_grps = SUPER_GRPS
    if "fine" in flags:
        sup_grps = (1, 1, 2, 2, 2, 2, 3, 3)
    use_accum = "na" not in flags
    split = "sp" in flags        # one exp per pair instead of per group
    stock_num = "sttn" in flags  # stock stt for num instead of custom amr
    stock_den = "tsd" in flags   # stock ts for den0 instead of amr+ones
    # split: exp accum straight to SBUF dg.  Forced if the compacted
    # key width leaves no spare tail column in the 2-bank PSUM tile
    # (cannot happen for the fixed-seed inputs, where padw = 928).
    sbuf_den = "sbd" in flags or padw >= 1024
    act_cp = "acp" in flags      # split: every 4th den extract on ACT
    # PSUM: merged = 2 group tiles of 4 banks; split = 4 pair tiles of
    # 2 banks.  The exp accum column lives in the tile's unused tail.
    assert split or gw < 2048, "padw too large for in-tile accum"
    supers = []
    g0 = 0
    for ng in sup_grps:
        if g0 >= ngrp:
            break
        ng = min(ng, ngrp - g0)
        supers.append((g0, g0 + ng))
        g0 += ng

    with tile.TileContext(nc) as tc:
        with (
            tc.tile_pool(name="consts", bufs=1) as consts,
            tc.tile_pool(name="epool", bufs=ep_bufs) as epool,
            tc.tile_pool(name="ps", bufs=(4 if split else 2),
                         space="PSUM") as ps,
        ):
            strm_bufs = [consts.tile([CROWS, NPAIR * padw], FP8,
                                     name=f"s{i}") for i in range(nbuf)]
            lhsT_bufs = [consts.tile([CROWS, 128], FP8, name=f"w{i}")
                         for i in range(nbuf)]
            yt_bufs = [consts.tile([128, B * padw], BF16, name=f"y{i}")
                       for i in range(nbuf)]
            res_bufs = [consts.tile([128, NPAIR], F32, name=f"r{i}")
                        for i in range(nbuf)]
            dg_bufs = [consts.tile([128, NPAIR], F32, name=f"dg{i}")
                       for i in range(nbuf)]
            d0_bufs = [consts.tile([128, NGRP], F32, name=f"d0{i}")
                       for i in range(nbuf)]
            scr = consts.tile([128, padw], BF16)
            scr2 = consts.tile([128, padw], BF16)
            ones = consts.tile([128, padw], BF16)
            warm = consts.tile([128, 1], F32)

            # preload the exp table set before the main loop
            nc.vector.memset(warm, 0.0)
            nc.scalar.activation(warm, warm,
                                 mybir.ActivationFunctionType.Exp)
            nc.vector.memset(ones, 1.0)
            if mode in ("dmaonly", "noexp", "nostt"):
                for r in res_bufs + dg_bufs + d0_bufs:
                    nc.vector.memset(r, 1.0)
            if mode == "nodma":
                for r in strm_bufs:
                    nc.vector.memset(r, 0.01)
                for r in lhsT_bufs:
                    nc.vector.memset(r, 0.01)
                for r in yt_bufs:
                    nc.vector.memset(r, 0.01)

            def num_op(e_sl, yt_b, acc):
                if stock_num:
                    nc.vector.scalar_tensor_tensor(
                        scr, e_sl, 0.0, yt_b,
                        op0=mybir.AluOpType.bypass,
                        op1=mybir.AluOpType.mult, accum_out=acc)
                else:
                    nc.vector.affine_mul_reduce(
                        out=scr, accum_out=acc, in0=e_sl, in1=yt_b,
                        scale=1.0, bias=0.0)

            def den_op(e_sl, acc):
                if stock_den:
                    nc.vector.tensor_scalar(
                        scr2, e_sl, 1.0, 0.0,
                        op0=mybir.AluOpType.mult,
                        op1=mybir.AluOpType.add, accum_out=acc)
                else:
                    nc.vector.affine_mul_reduce(
                        out=scr2, accum_out=acc, in0=e_sl, in1=ones,
                        scale=1.0, bias=0.0)

            def merged_group(strm, lhsT, yt, res, dg, d0, g):
                b = g // (ngrp // B)
                col0 = g * gw
                l_ps = ps.tile([128, 2048], F32, tag="l")
                for hh in range(0, gw, 512):
                    he = min(hh + 512, gw)
                    nc.tensor.matmul(
                        l_ps[:, hh:he], lhsT=lhsT,
                        rhs=strm[:, col0 + hh:col0 + he],
                        start=True, stop=True)
                if mode == "noexp":
                    return
                e_t = epool.tile([128, gw], BF16, tag="e")
                nc.scalar.activation(
                    e_t, l_ps[:, 0:gw],
                    mybir.ActivationFunctionType.Exp,
                    accum_out=(l_ps[:, 2047:2048] if use_accum else None))
                if mode == "nostt":
                    return
                # extract first: it releases the PSUM tile for the
                # pipelined matmuls two groups ahead
                if use_accum:
                    nc.vector.tensor_copy(dg[:, g:g + 1],
                                          l_ps[:, 2047:2048])
                yt_b = yt[:, b * padw:(b + 1) * padw]
                num_op(e_t[:, 0:padw], yt_b, res[:, 2 * g:2 * g + 1])
                num_op(e_t[:, padw:gw], yt_b, res[:, 2 * g + 1:2 * g + 2])
                den_op(e_t[:, 0:padw], d0[:, g:g + 1])

            def split_pair(strm, lhsT, yt, res, dg, pp):
                b = pp // (npair // B)
                col0 = pp * padw
                l_ps = ps.tile([128, 1024], F32, tag="l")
                for hh in range(0, padw, 512):
                    he = min(hh + 512, padw)
                    nc.tensor.matmul(
                        l_ps[:, hh:he], lhsT=lhsT,
                        rhs=strm[:, col0 + hh:col0 + he],
                        start=True, stop=True)
                if mode == "noexp":
                    return
                e_t = epool.tile([128, padw], BF16, tag="e")
                acc = None
                if use_accum:
                    acc = dg[:, pp:pp + 1] if sbuf_den else l_ps[:, 1023:1024]
                nc.scalar.activation(
                    e_t, l_ps[:, 0:padw],
                    mybir.ActivationFunctionType.Exp,
                    accum_out=acc)
                if mode == "nostt":
                    return
                if use_accum and not sbuf_den:
                    if act_cp and pp % 4 == 3:
                        nc.scalar.activation(
                            dg[:, pp:pp + 1], l_ps[:, 1023:1024],
                            mybir.ActivationFunctionType.Copy)
                    else:
                        nc.vector.tensor_copy(dg[:, pp:pp + 1],
                                              l_ps[:, 1023:1024])
                yt_b = yt[:, b * padw:(b + 1) * padw]
                num_op(e_t, yt_b, res[:, pp:pp + 1])

            def main_pass(strm, lhsT, yt, res, dg, d0):
                if mode != "nodma":
                    nc.sync.dma_start(lhsT, lhsT_d)
                    nc.sync.dma_start(yt, yt_d)
                    for (q0, q1) in supers:
                        nc.sync.dma_start(
                            strm[:, q0 * gw:q1 * gw],
                            strm_d[:, q0 * gw:q1 * gw])
                if mode != "dmaonly":
                    if split:
                        for pp in range(npair):
                            split_pair(strm, lhsT, yt, res, dg, pp)
                    else:
                        for g in range(ngrp):
                            merged_group(strm, lhsT, yt, res, dg, d0, g)
                if "noo" not in flags:
                    # layout: merged [dgrp 16 | d0 16 | num 32],
                    #         split  [den 32 | num 32]
                    if split:
                        nc.sync.dma_start(out_d[:, 0:NPAIR],
                                          dg[:, 0:NPAIR])
                    else:
                        nc.sync.dma_start(out_d[:, 0:NGRP],
                                          dg[:, 0:NGRP])
                        nc.sync.dma_start(out_d[:, NGRP:2 * NGRP], d0)
                    nc.sync.dma_start(out_d[:, NPAIR:2 * NPAIR], res)

            if loop_reps > 1:
                if nbuf == 2:
                    assert loop_reps % 2 == 0, "loop_reps must be even"
                with tc.For_i(0, loop_reps // nbuf, 1,
                              hint_engines=(mybir.EngineType.PE,
                                            mybir.EngineType.Activation,
                                            mybir.EngineType.DVE,
                                            mybir.EngineType.Pool,
                                            mybir.EngineType.SP)):
                    for ib in range(nbuf):
                        main_pass(strm_bufs[ib], lhsT_bufs[ib],
                                  yt_bufs[ib], res_bufs[ib],
                                  dg_bufs[ib], d0_bufs[ib])
            else:
                main_pass(strm_bufs[0], lhsT_bufs[0], yt_bufs[0],
                          res_bufs[0], dg_bufs[0], d0_bufs[0])

    nc.compile()
    return nc


def _build_microbench(nc, out_d, mode, loop_reps):
    """Pure per-engine instruction pacing benches: NI dependency-free
    instructions per iteration on one engine (same-engine WAW only)."""
    NI = 128
    width = 2048 if "2048" in mode else (1024 if "1024" in mode else 512)
    accum = "na" not in mode
    edt = mybir.dt.float8e4 if "f8" in mode else BF16
    accum_psum = mode.endswith("p")
    with tile.TileContext(nc) as tc:
        with (
            tc.tile_pool(name="consts", bufs=1) as consts,
            tc.tile_pool(name="epool", bufs=4) as epool,
            tc.tile_pool(name="ps", bufs=2, space="PSUM") as ps,
            tc.tile_pool(name="psc", bufs=1, space="PSUM") as psc,
        ):
            g_all = consts.tile([128, 4096], BF16)
            den_buf = consts.tile([128, NBLK], F32)
            num_buf = consts.tile([128, NBLK], F32)
            scr = consts.tile([128, width], edt)
            e_src = consts.tile([128, width], edt)
            warm = consts.tile([128, 1], F32)
            nc.vector.memset(warm, 0.0)
            nc.scalar.activation(warm, warm,
                                 mybir.ActivationFunctionType.Exp)
            nc.vector.memset(g_all, 0.01)
            nc.vector.memset(e_src, 1.0)
            nc.vector.memset(den_buf, 1.0)
            nc.vector.memset(num_buf, 1.0)
            l_ps = psc.tile([128, width], F32)
            if accum_psum:
                den_ps = psc.tile([128, NBLK], F32)
            for h in range(0, width, 512):
                nc.tensor.matmul(l_ps[:, h:h + 512], lhsT=g_all[:, 0:128],
                                 rhs=g_all[:, 128 + h:640 + h],
                                 start=True, stop=True)

            def body():
                for k in range(NI):
                    if mode.startswith("actb"):
                        e_t = epool.tile([128, width], edt, tag="e")
                        tgt = den_ps if accum_psum else den_buf
                        nc.scalar.activation(
                            e_t, l_ps, mybir.ActivationFunctionType.Exp,
                            accum_out=(tgt[:, k % NBLK:k % NBLK + 1]
                                       if accum else None))
                    elif mode.startswith("dveb"):
                        if "ts" in mode:
                            nc.vector.tensor_scalar(
                                scr, e_src, 1.0, 0.0,
                                op0=mybir.AluOpType.mult,
                                op1=mybir.AluOpType.add,
                                accum_out=num_buf[:, k % NBLK:k % NBLK + 1])
                        else:
                            nc.vector.scalar_tensor_tensor(
                                scr, e_src, 0.0, g_all[:, 128:128 + width],
                                op0=mybir.AluOpType.bypass,
                                op1=mybir.AluOpType.mult,
                                accum_out=num_buf[:, k % NBLK:k % NBLK + 1])
                    elif mode.startswith("redb"):
                        nc.vector.tensor_reduce(
                            num_buf[:, k % NBLK:k % NBLK + 1], e_src,
                            axis=mybir.AxisListType.X,
                            op=mybir.AluOpType.add)
                    elif mode.startswith("ttrb"):
                        nc.vector.tensor_tensor_reduce(
                            scr, e_src, g_all[:, 128:128 + width],
                            1.0, 0.0,
                            op0=mybir.AluOpType.mult,
                            op1=mybir.AluOpType.add,
                            accum_out=num_buf[:, k % NBLK:k % NBLK + 1])
                    elif mode.startswith("ttb"):
                        nc.vector.tensor_tensor(
                            scr, e_src, g_all[:, 128:128 + width],
                            op=mybir.AluOpType.mult)
                    elif mode.startswith("amrb"):
                        nc.vector.affine_mul_reduce(
                            out=scr, in0=e_src,
                            in1=g_all[:, 128:128 + width],
                            scale=1.0, bias=0.0,
                            accum_out=num_buf[:, k % NBLK:k % NBLK + 1])
                    elif mode.startswith("poolts"):
                        nc.gpsimd.tensor_scalar(
                            scr, e_src, 1.0, 0.0,
                            op0=mybir.AluOpType.mult,
                            op1=mybir.AluOpType.add,
                            accum_out=num_buf[:, k % NBLK:k % NBLK + 1])
                    elif mode.startswith("pooltt"):
                        nc.gpsimd.tensor_tensor(
                            scr, e_src, g_all[:, 128:128 + width],
                            op=mybir.AluOpType.mult)
                    elif mode.startswith("poolb"):
                        nc.gpsimd.scalar_tensor_tensor(
                            scr, e_src, 0.0, g_all[:, 128:128 + width],
                            op0=mybir.AluOpType.bypass,
                            op1=mybir.AluOpType.mult,
                            accum_out=num_buf[:, k % NBLK:k % NBLK + 1])
                    else:  # mmb
                        o = ps.tile([128, 512], F32, tag="l")
                        nc.tensor.matmul(o, lhsT=g_all[:, 0:128],
                                         rhs=g_all[:, 128:640],
                                         start=True, stop=True)
                nc.sync.dma_start(out_d[:, 0:NPAIR], den_buf[:, 0:NPAIR])
                nc.sync.dma_start(out_d[:, NPAIR:2 * NPAIR],
                                  num_buf[:, 0:NPAIR])

            if loop_reps > 1:
                with tc.For_i(0, loop_reps, 1,
                              hint_engines=(mybir.EngineType.PE,
                                            mybir.EngineType.Activation,
                                            mybir.EngineType.DVE,
                                            mybir.EngineType.SP)):
                    body()
            else:
                body()
    nc.compile()
    return nc


def _get_program(nblk=NBLK, loop_reps=1, mode="full"):
    key = ("nc", nblk, loop_reps, mode, _LAYOUT["padw"])
    if key not in _PROGRAM_CACHE:
        _PROGRAM_CACHE[key] = _build_program(nblk, loop_reps, mode)
    return _PROGRAM_CACHE[key]


def _host_prep(pairwise_g, coset_functions, mask, w_y, w_g):
    """Build the per-core fp8/bf16 input tensors with a mask-compacted
    key axis.

    The free (key) axis is the flat (s2, j) list of mask-valid columns per
    batch, padded to a static width padw; pad columns carry L = -240 so
    their exp is ~0 (<=2e-6 after the 1/16 indicator).  Order is
    irrelevant (den/num are plain sums)."""
    y = np.asarray(coset_functions, dtype=np.float32)    # (B, N, S, C)
    maskb = np.asarray(mask)
    w_y0 = np.asarray(w_y, dtype=np.float32)[:, 0]
    w_g = np.asarray(w_g, dtype=np.float32)

    # valid flat key indices v = s2*N + j, per batch
    mflat = maskb.transpose(0, 2, 1).reshape(B, S * N)
    counts = mflat.sum(axis=1)
    padw = max(32, int(-(-int(counts.max()) // 32) * 32))
    _LAYOUT.update(padw=padw)

    gidx = np.zeros((B, padw), np.int64)
    pad = np.ones((B, padw), bool)
    for b in range(B):
        ix = np.flatnonzero(mflat[b])
        gidx[b, :len(ix)] = ix
        pad[b, :len(ix)] = False

    # ycols[b, v, c] = y[b, j(v), s2(v), c]
    yv = y.transpose(0, 2, 1, 3).reshape(B, S * N, CIN)
    ycols = np.stack([yv[b, gidx[b]] for b in range(B)])  # (B, padw, C)
    # L rows (C, padw) *LSCALE: w_y0*y on valid cols, -240 on pads
    ld = LSCALE * (w_y0 * ycols)
    ld[pad] = -240.0
    ld = np.ascontiguousarray(ld.transpose(0, 2, 1))      # (B, C, padw)
    # y table, zeroed on pads, duplicated over (i2, s1)
    ytab = np.where(pad[..., None], 0.0, ycols).transpose(0, 2, 1)
    yt = np.broadcast_to(ytab[:, None], (B, 16, CIN, padw))
    yt_plane = yt.reshape(B, 128, padw).transpose(1, 0, 2).reshape(128, -1)
    yt_plane = np.ascontiguousarray(yt_plane, dtype=NPBF16)

    # stationary lhsT (120, 128): out col k = i2*64 + s1*8 + c
    lhsT = np.zeros((CROWS, 128), np.float32)
    for i2 in range(2):
        for s1 in range(S):
            for g in range(GDIM):
                p = i2 * 56 + s1 * GDIM + g
                k0 = i2 * 64 + s1 * 8
                lhsT[p, k0:k0 + CIN] = w_g[:, g]
    for c in range(CIN):
        for i2 in range(2):
            for s1 in range(S):
                lhsT[112 + c, i2 * 64 + s1 * 8 + c] = 1.0 / LSCALE
    lhsT = np.ascontiguousarray(lhsT, dtype=NPF8)

    pairwise_g = np.asarray(pairwise_g, dtype=np.float32)
    in_maps = []
    for k in range(NCORES):
        sl = slice(ISHARD * k, ISHARD * (k + 1))
        pg = pairwise_g[:, sl]                   # (B, 16, j, s1, s2, g)
        x = pg.reshape(B, 8, 2, N, S, S, GDIM)
        x = x.transpose(0, 1, 2, 4, 6, 5, 3)     # [b,ip,i2,s1,g,s2,j]
        x = np.ascontiguousarray(x).reshape(B, 8, 2, S, GDIM, S * N)
        xg = np.stack([x[b][..., gidx[b]] for b in range(B)])
        xg = xg.reshape(NPAIR, 112, padw)
        strm = np.empty((CROWS, NPAIR, padw), NPF8)
        strm[0:112] = xg.transpose(1, 0, 2)
        strm[112:CROWS] = np.repeat(ld, NPAIR // B, axis=0) \
            .transpose(1, 0, 2)
        in_maps.append({"strm": strm.reshape(CROWS, -1),
                        "lhsT": lhsT, "yt": yt_plane})
    return in_maps


def _host_finish(s_list, coset_functions, mask, w_lin,
                 split=("sp" in DEFAULT_MODE)):
    """Decode per-core (128, 64) outputs into the full result."""
    y = np.asarray(coset_functions, dtype=np.float32)
    maskf = np.asarray(mask).astype(np.float32)
    w_lin = np.asarray(w_lin, dtype=np.float32)
    out = np.empty((B, N, S, COUT), np.float32)
    for k in range(NCORES):
        s = np.asarray(s_list[k], dtype=np.float32)      # (128, 64)
        num = s[:, NPAIR:]
        if split:
            den = s[:, 0:NPAIR].copy()
        else:
            dgrp = s[:, 0:NGRP]
            d0 = s[:, NGRP:2 * NGRP]
            den = np.empty((128, NPAIR), np.float32)
            den[:, 0::2] = d0
            den[:, 1::2] = dgrp - d0
        # partition p = i2*64 + s1*8 + c; block = 2*pp + i2
        den = den.reshape(2, S, CIN, NPAIR).transpose(3, 0, 1, 2)
        num = num.reshape(2, S, CIN, NPAIR).transpose(3, 0, 1, 2)
        den = den.reshape(NBLK, S, CIN)
        num = num.reshape(NBLK, S, CIN)
        sl = slice(ISHARD * k, ISHARD * (k + 1))
        y_q = y[:, sl].reshape(NBLK, S, CIN)
        m_q = maskf[:, sl].reshape(NBLK, S)
        res = (y_q + num / den) * m_q[..., None]
        res = res @ w_lin.T
        out[:, sl] = res.reshape(B, ISHARD, S, COUT)
    return out


def kernel(pairwise_g, coset_functions, mask, w_y, b_y, w_g, b_g, w_lin):
    pairwise_g = np.asarray(pairwise_g)
    coset_functions = np.asarray(coset_functions)
    mask = np.asarray(mask)

    in_maps = _host_prep(pairwise_g, coset_functions, mask,
                         np.asarray(w_y), np.asarray(w_g))
    nc = _get_program(NBLK, 1, DEFAULT_MODE)
    res = run_bass_kernel_spmd(nc, in_maps, core_ids=list(range(NCORES)))
    s_list = [r["out_s"] for r in res.results]
    return _host_finish(s_list, coset_functions, mask, np.asarray(w_lin))
